# revision 36
# baseline (speedup 1.0000x reference)
"""Trainium2 Bass kernel for CapsuleLayer dynamic routing (fp16, pipelined).

Math (reference):
    u_hat[b,i,j,e] = sum_d inputs[b,i,d] * kernel[i,j,d,e]
    3 routing iterations over shared bias[i,j] (softmax over j),
    s[b,j,e] = sum_i c[i,j] u_hat[b,i,j,e]; outputs = squash(s)
    bias += sum_{b,e} u_hat * outputs

Strategy: shard i (in_caps=1152 -> 144/core) across 8 cores; u_hat never
materialized.  All data SBUF-resident in fp16.  K stored with (e,j)
innermost so the c[i,j] broadcast lands on a middle dim (DVE 2x mode
needs unit-stride last dims).

Per routing iteration (A = chunks 0..7, B = 8..15, tails 16,17 first):
    per chunk: G = X^T O (PE, psum) -> fp16 copy (Act) -> P = K (*) G
    (DVE fp16 2x) -> d-sum on PE into row-halves of one psum tile Qt
    (A -> rows 0..63 via selfull[:, c, :64], B -> rows 64..127).
    Tail chunks take a DVE e-reduce + sel8-matmul path and are processed
    first so their serial reduces hide under the main pipeline.
    A's half closes right after chunk 8's G/P (e-reduce A -> bmask
    replicate -> bias/softmax for chunk-cols 0..7 -> cK-A scale) so the
    s matmuls for chunks 0..7 run right after the B d-sum burst while
    the B softmax + cK-B scale are still on Act/DVE.  B's Q matmuls
    trail their P by one chunk and keep bmA ahead of the B group-open.
    s accumulates into Qt rows 0..B (A data dead, B rows read by itB).
    AllReduce(s) fp16; squash -> O.  A junk matmul gated on s_full
    re-ramps the PE out of its idle pstate during each squash.
Final iteration: ReduceScatter, each core squashes + emits 8 batches.

The softmax over j is per (i, chunk-col) so the A/B split is exact.
NOTE pool_ck=True (cK scale groups on gpsimd) is OFF: the CoreSim cost
model prices Pool tensor_tensor at ~0.83ns/elem but on HW it measured
~16us/iter SLOWER (within-run A/B 189.8us vs 157.4us) — gpsimd is not
usable for bulk elementwise work.

Measured (HW repeat-delta, within one process): legacy 184,675 ns ->
this version 157,376 ns; standalone test.py run: 164,617 ns, rel err
2.291e-3 (gate 2e-2).  Cross-process numbers carry ~5-10us of
dispatch-anchor noise; only within-run comparisons are reliable.
"""

import sys

import numpy as np

if "/opt/trn_rl_repo" not in sys.path:
    sys.path.insert(0, "/opt/trn_rl_repo")

B, I, D, J, E = 64, 1152, 16, 32, 32
N_CORES = 8
I_LOC = I // N_CORES            # 144
ID = I_LOC * D                  # 2304
NCHUNK = ID // 128              # 18
NMAIN = 16                      # d-sum-first chunks (full-stationary masks)
NA = 8                          # A-half main chunks (0..7); B = 8..15 + tails
ISUB = 128 // D                 # 8 distinct i per 128-row chunk
JE = J * E                      # 1024
BSH = B // N_CORES              # 8 output batches per core
EPS = 1e-7
ROUTING_STEPS = 2               # routing iters after the uniform-c step

_CACHE = {}
AR_F32 = False  # fp16 collectives halve AR payload


def _build_nc(repeat=1, comm=True, ar_f32=False, pool_ck=False,
              direct_p=()):
    import concourse.mybir as mybir
    import concourse.tile as tile
    from concourse import bacc

    f32 = mybir.dt.float32
    f16 = mybir.dt.float16
    AX = mybir.AxisListType
    OP = mybir.AluOpType
    AF = mybir.ActivationFunctionType

    nc = bacc.Bacc("TRN2", target_bir_lowering=False, debug=False,
                   num_devices=N_CORES)
    x_d = nc.dram_tensor("x", [B, ID], f16, kind="ExternalInput")
    xt_d = nc.dram_tensor("xt", [ID, B], f16, kind="ExternalInput")
    k_d = nc.dram_tensor("kk", [ID, JE], f16, kind="ExternalInput")
    # packed constants: [sel8(128) | selfull(16*128) | bmask(16*128)]
    cst_d = nc.dram_tensor("cst", [128, 4224], f16, kind="ExternalInput")
    out_d = nc.dram_tensor("out", [BSH, JE], f32, kind="ExternalOutput")
    fAR = f32 if ar_f32 else f16
    arin_d = nc.dram_tensor("ar_in", [B, JE], fAR)
    arout_d = nc.dram_tensor("ar_out", [B, JE], fAR, addr_space="Shared")
    rsout_d = nc.dram_tensor("rs_out", [BSH, JE], fAR)
    RG = [list(range(N_CORES))]

    # cK scale groups; "pool" groups ride the otherwise-idle gpsimd
    # (sim Pool TT ~0.83ns/elem vs DVE-2x 0.55 — worth a big share)
    if pool_ck:
        CK_A = [([0, 1], "dve"), ([2, 3, 4], "pool"), ([5, 6, 7], "pool")]
        CK_B = [([8, 9], "dve"), ([10, 11], "dve"), ([12, 13], "dve"),
                ([14, 15, 16, 17], "pool")]
    else:
        CK_A = [([0, 1], "dve"), ([2, 3, 4], "dve"), ([5, 6, 7], "dve")]
        # B groups split at chunk 12: softmax for cols 8..11 closes first
        # so cK [8,9]/[10,11] start while cols 12..17 still in exp/sum
        CK_B = [([8, 9], "dve"), ([10, 11], "dve"), ([12, 13], "dve"),
                ([14, 15], "dve"), ([16, 17], "dve")]

    with tile.TileContext(nc) as tc:
        with (
            tc.tile_pool(name="big", bufs=1) as big,
            tc.tile_pool(name="work", bufs=2) as work,
            tc.tile_pool(name="pwork", bufs=1) as pwork,
            tc.tile_pool(name="gwork", bufs=6) as gwork,
            tc.tile_pool(name="once", bufs=1) as once,
            tc.tile_pool(name="small", bufs=2) as small,
            tc.tile_pool(name="gps", bufs=2, space="PSUM") as gps,
            tc.tile_pool(name="sps", bufs=1, space="PSUM") as sps,
            tc.tile_pool(name="qps", bufs=1, space="PSUM") as qps,
        ):
            # ---- resident inputs; few big DMAs (SP issuance ~600ns each)
            ksb = big.tile([128, NCHUNK, JE], f16)
            xtsb = big.tile([128, NCHUNK, B], f16)
            xsb = big.tile([B, ID], f16)
            cst = big.tile([128, 4224], f16)
            nc.sync.dma_start(
                xtsb[:], xt_d[:].rearrange("(c p) n -> p c n", p=128))
            nc.sync.dma_start(xsb[:], x_d[:])
            for g in range(3):
                nc.sync.dma_start(
                    ksb[:, g * 6:(g + 1) * 6, :],
                    k_d[:].rearrange("(c p) n -> p c n",
                                     p=128)[:, g * 6:(g + 1) * 6, :])
            nc.sync.dma_start(cst[:], cst_d[:])
            sel8 = cst[:, 0:128]
            selfull = cst[:, 128:2176].rearrange("p (m q) -> p m q", q=128)
            bmask = cst[:, 2176:4224].rearrange("p (m q) -> p m q", q=128)

            epsb = big.tile([B, 1], f32)
            nc.vector.memset(epsb[:], EPS)

            # preload the one act table serving copy/square/ln/exp so the
            # greedy per-func table chooser stops thrashing (~1.4us/load)
            import bass_rust
            _atl = bass_rust.InstLoadActFuncSet(
                name=nc.get_next_instruction_name(),
                act_func_set_id=6, ins=[], outs=[])
            nc.scalar.add_instruction(_atl)

            bias = big.tile([128, NCHUNK, J], f32)
            crep = big.tile([128, NCHUNK, J], f16)
            s_full = big.tile([B, JE], fAR)
            orr = big.tile([B, JE], f16)

            def emit_squash(alpha, nb, s_in, final):
                # squash(alpha * s_in) over e; layout [(b), (e j)]
                # square on Act (DVE is the busier engine); alpha**2 folded
                # into the tiny per-j n2 when alpha != 1
                # s*s on DVE fp16 2x: shortest serial chain (this path has
                # every engine idle, so chain length beats engine balance)
                sq = once.tile([B, JE], f16, tag="sq")
                with nc.allow_low_precision("fp16 squash"):
                    nc.vector.tensor_tensor(sq[:nb, :], s_in[:nb, :],
                                            s_in[:nb, :], op=OP.mult)
                m2 = small.tile([B, J], f32, tag="m2")
                nc.vector.tensor_reduce(
                    m2[:nb, :],
                    sq[:nb, :].rearrange("b (e j) -> b j e", j=J),
                    axis=AX.X, op=OP.add)
                if alpha != 1.0:
                    n2 = small.tile([B, J], f32, tag="n2")
                    nc.vector.tensor_scalar_mul(n2[:nb, :], m2[:nb, :],
                                                alpha * alpha)
                else:
                    n2 = m2
                # 1/sqrt(n2+eps) = exp(-0.5*ln(n2+eps)): keeps Act inside
                # one function table (exp/ln/copy/square)
                lg = small.tile([B, J], f32, tag="lg")
                nc.scalar.activation(lg[:nb, :], n2[:nb, :], AF.Ln,
                                     bias=epsb[:nb, :])
                rsq = small.tile([B, J], f32, tag="rsq")
                nc.scalar.activation(rsq[:nb, :], lg[:nb, :], AF.Exp,
                                     scale=-0.5)
                d1 = small.tile([B, J], f32, tag="d1")
                nc.vector.tensor_scalar_add(d1[:nb, :], n2[:nb, :], 1.0)
                rcp = small.tile([B, J], f32, tag="rcp")
                nc.vector.reciprocal(rcp[:nb, :], d1[:nb, :])
                if alpha != 1.0:
                    n2s = small.tile([B, J], f32, tag="n2s")
                    nc.vector.tensor_scalar_mul(n2s[:nb, :], n2[:nb, :],
                                                alpha)
                else:
                    n2s = n2
                fac0 = small.tile([B, J], f32, tag="fac0")
                nc.vector.tensor_tensor(fac0[:nb, :], n2s[:nb, :],
                                        rsq[:nb, :], op=OP.mult)
                # fp16 factor -> the big output multiply runs in DVE 2x mode
                facf = small.tile([B, J], f16, tag="facf")
                with nc.allow_low_precision("fp16 squash"):
                    nc.vector.tensor_tensor(facf[:nb, :], fac0[:nb, :],
                                            rcp[:nb, :], op=OP.mult)
                fb = facf[:nb, None, :].broadcast_to([nb, E, J])
                s3 = s_in[:nb, :].rearrange("b (e j) -> b e j", j=J)
                if final:
                    osb = once.tile([BSH, JE], f32, tag="osb")
                    with nc.allow_low_precision("fp16 squash"):
                        nc.vector.tensor_tensor(
                            osb[:].rearrange("b (j e) -> b e j", e=E),
                            s3, fb, op=OP.mult)
                    nc.sync.dma_start(out_d[:], osb[:])
                else:
                    with nc.allow_low_precision("fp16 squash"):
                        nc.vector.tensor_tensor(
                            orr[:nb, :].rearrange("b (e j) -> b e j", j=J),
                            s3, fb, op=OP.mult)

            def emit_evac(s_ps):
                # psum -> SBUF -> arin_d in column halves: half 0's DMA
                # overlaps half 1's Act copy
                s_sb = once.tile([B, JE], fAR, tag="s_sb")
                for h in range(2):
                    sl = slice(h * 512, (h + 1) * 512)
                    nc.scalar.activation(s_sb[:, sl], s_ps[:, sl], AF.Copy)
                    nc.sync.dma_start(arin_d[:, sl], s_sb[:, sl])

            def emit_allreduce(s_ps):
                emit_evac(s_ps)
                if comm:
                    nc.gpsimd.collective_compute(
                        "AllReduce", OP.add, replica_groups=RG,
                        ins=[arin_d[:]], outs=[arout_d[:]])
                    nc.sync.dma_start(s_full[:], arout_d[:])
                else:
                    nc.sync.dma_start(s_full[:], arin_d[:])

            def emit_pe_warmup():
                # PE drops to low pstate during the collective; a junk
                # matmul gated on s_full's arrival ramps it back up so the
                # first real G matmuls run at speed. Output is discarded
                # (scratch psum bank reused each time).
                wps = gps.tile([128, JE], f32, tag="g")
                for h in range(2):
                    nc.tensor.matmul(
                        wps[:, h * 512:(h + 1) * 512],
                        s_full[:, 0:128], s_full[:, h * 512:(h + 1) * 512],
                        start=True, stop=True)

            def emit_s_matmuls(s_ps, rhs_of_chunk, chunks, start, stop):
                for n, c in enumerate(chunks):
                    rhs = rhs_of_chunk(c)
                    for h in range(2):
                        nc.tensor.matmul(
                            s_ps[0:B, h * 512:(h + 1) * 512],
                            xtsb[:, c, :],
                            rhs[:, h * 512:(h + 1) * 512],
                            start=(start and n == 0),
                            stop=(stop and n == len(chunks) - 1))

            for _rep in range(repeat):
                # ---- phase 0: s0 = X @ K (uniform c folded via alpha=1/J)
                s_ps = qps.tile([128, JE], f32, tag="qt")
                emit_s_matmuls(s_ps, lambda c: ksb[:, c, :],
                               list(range(NCHUNK)), True, True)
                emit_allreduce(s_ps[0:B, :])
                emit_squash(1.0 / J, B, s_full, final=False)

                for r in range(ROUTING_STEPS):
                    emit_pe_warmup()
                    qt = qps.tile([128, JE], f32, tag="qt")
                    incr_ps = sps.tile([128, NCHUNK * J], f32, tag="incr")
                    exe = once.tile([128, NCHUNK, J], f32, tag="exe")
                    it2 = once.tile([128, J], f16, tag="it2")
                    sm = small.tile([128, NCHUNK], f32, tag="sm")
                    rc = small.tile([128, NCHUNK], f32, tag="rc")
                    kps = {}
                    all_groups = ([(tuple(g), eng, f"kpA{n}")
                                   for n, (g, eng) in enumerate(CK_A)] +
                                  [(tuple(g), eng, f"kpB{n}")
                                   for n, (g, eng) in enumerate(CK_B)])
                    NGA = len(CK_A)

                    def scale_group(gi, _kps=kps):
                        grp, eng, tg = all_groups[gi]
                        if grp in _kps:
                            return
                        w, lo = len(grp), grp[0]
                        kp = work.tile([128, w, JE], f16, tag=tg)
                        engine = nc.vector if eng == "dve" else nc.gpsimd
                        with nc.allow_low_precision("fp16 cK"):
                            engine.tensor_tensor(
                                kp[:].rearrange("p c (e j) -> p c e j", j=J),
                                ksb[:, lo:lo + w, :].rearrange(
                                    "p c (e j) -> p c e j", j=J),
                                crep[:, lo:lo + w, None, :]
                                .broadcast_to([128, w, E, J]),
                                op=OP.mult)
                        _kps[grp] = kp

                    def scaled_k(c, _kps=kps):
                        for grp, eng, tg in all_groups:
                            if c in grp:
                                return _kps[grp][:, c - grp[0], :]
                        raise AssertionError(c)

                    def emit_softmax_half(cols, r):
                        # cols: (lo, hi) chunk-col range of bias/softmax
                        lo, hi = cols
                        bfl = bias[:].rearrange("p c j -> p (c j)")
                        if r > 0:
                            nc.vector.tensor_tensor(
                                bfl[:, lo * J:hi * J],
                                bfl[:, lo * J:hi * J],
                                incr_ps[:, lo * J:hi * J], op=OP.add)
                            src = bias[:, lo:hi, :]
                        else:
                            # exp straight off psum; bias saved later (DVE)
                            src = incr_ps[:, lo * J:hi * J].rearrange(
                                "p (c j) -> p c j", j=J)
                        nc.scalar.activation(exe[:, lo:hi, :], src, AF.Exp)
                        nc.vector.tensor_reduce(
                            sm[:, lo:hi], exe[:, lo:hi, :], axis=AX.X,
                            op=OP.add)
                        nc.vector.reciprocal(rc[:, lo:hi], sm[:, lo:hi])
                        with nc.allow_low_precision("fp16 c"):
                            nc.vector.tensor_tensor(
                                crep[:, lo:hi, :], exe[:, lo:hi, :],
                                rc[:, lo:hi, None]
                                .broadcast_to([128, hi - lo, J]),
                                op=OP.mult)

                    def emit_close_A():
                        # e-reduce rows 0..63, replicate via bmask,
                        # softmax for chunk-cols 0..7, first cK-A group.
                        # Emitted after main chunk 9 so neither PE (bmA
                        # waits itA) nor Act (expA waits bmA) stalls.
                        with nc.allow_low_precision("fp16 incr"):
                            nc.vector.tensor_reduce(
                                it2[0:64, :],
                                qt[0:64, :].rearrange(
                                    "p (e j) -> p j e", j=J),
                                axis=AX.X, op=OP.add)
                        for cc in range(NA):
                            nc.tensor.matmul(
                                incr_ps[:, cc * J:(cc + 1) * J],
                                bmask[0:64, cc, :], it2[0:64, :],
                                start=True, stop=True)
                        emit_softmax_half((0, NA), r)
                        for gi in range(NGA):
                            scale_group(gi)   # pool group runs concurrent

                    # ---- increments: tails first, then A (0..7), B (8..15)
                    # DIRECT_P chunks skip the Act copy: DVE multiplies K
                    # against the fp32 psum G directly (1x mode) — trades
                    # idle DVE for Act-pipe pace, and {14,15} unhook the
                    # pipe tail from the Act queue so itB closes earlier.
                    DIRECT_P = set(direct_p)

                    def emit_gcp(c):
                        # G = X^T O (PE) -> [fp16 copy (Act) ->] P (DVE)
                        g_ps = gps.tile([128, JE], f32, tag="g")
                        for h in range(2):
                            nc.tensor.matmul(
                                g_ps[:, h * 512:(h + 1) * 512],
                                xsb[:, c * 128:(c + 1) * 128],
                                orr[:, h * 512:(h + 1) * 512],
                                start=True, stop=True)
                        psup = pwork.tile([128, JE], f16, tag=f"p{c}")
                        if c in DIRECT_P:
                            nc.vector.tensor_tensor(psup[:], ksb[:, c, :],
                                                    g_ps[:], op=OP.mult)
                        else:
                            g_sb = gwork.tile([128, JE], f16, tag="gsb")
                            nc.scalar.activation(g_sb[:], g_ps[:], AF.Copy)
                            nc.vector.tensor_tensor(psup[:], ksb[:, c, :],
                                                    g_sb[:], op=OP.mult)
                        return psup

                    pend_sel8 = []
                    for c in [16, 17] + list(range(NA)):
                        psup = emit_gcp(c)
                        if c >= NMAIN:  # tail chunk: e-reduce + sel8 later
                            per = small.tile([128, J], f16, tag=f"per{c}")
                            with nc.allow_low_precision("fp16 incr"):
                                nc.vector.tensor_reduce(
                                    per[:],
                                    psup[:].rearrange(
                                        "p (e j) -> p j e", j=J),
                                    axis=AX.X, op=OP.add)
                            pend_sel8.append((c, per))
                            continue
                        for h in range(2):
                            nc.tensor.matmul(
                                qt[0:64, h * 512:(h + 1) * 512],
                                selfull[:, c, 0:64],
                                psup[:, h * 512:(h + 1) * 512],
                                start=(c == 0), stop=(c == NA - 1))
                        if c == 3 and pend_sel8:
                            # tails' incr cols; PE slack mid-pipe
                            for tc_, per in pend_sel8:
                                nc.tensor.matmul(
                                    incr_ps[:, tc_ * J:(tc_ + 1) * J],
                                    sel8[:], per[:],
                                    start=True, stop=True)
                            pend_sel8 = []
                    # B half: G/copy/P stream; close-A (incl. the itA read
                    # of qt) lands right after P8 so bmA precedes Q8's
                    # group-open in PE order; Q_c trails by one chunk.
                    prev_psup = None
                    for c in range(NA, NMAIN):
                        psup = emit_gcp(c)
                        if c == NA:
                            emit_close_A()
                        else:
                            for h in range(2):
                                nc.tensor.matmul(
                                    qt[64:128, h * 512:(h + 1) * 512],
                                    selfull[:, c - 1, 64:128],
                                    prev_psup[:, h * 512:(h + 1) * 512],
                                    start=(c - 1 == NA), stop=False)
                        prev_psup = psup
                    for h in range(2):
                        nc.tensor.matmul(
                            qt[64:128, h * 512:(h + 1) * 512],
                            selfull[:, NMAIN - 1, 64:128],
                            prev_psup[:, h * 512:(h + 1) * 512],
                            start=False, stop=True)
                    # close B: e-reduce rows 64..127, replicate, softmax
                    # cols 8..17 (tails' cols were filled via sel8)
                    with nc.allow_low_precision("fp16 incr"):
                        nc.vector.tensor_reduce(
                            it2[64:128, :],
                            qt[64:128, :].rearrange(
                                "p (e j) -> p j e", j=J),
                            axis=AX.X, op=OP.add)
                    for cc in range(NA, NMAIN):
                        nc.tensor.matmul(
                            incr_ps[:, cc * J:(cc + 1) * J],
                            bmask[64:128, cc, :], it2[64:128, :],
                            start=True, stop=True)
                    # ---- s_{r+1} = X @ (c (x) K): s reuses qt rows 0..B
                    # (A-group data is dead, B-rows already read by itB —
                    # bmB precedes sA in PE order so the group-open is
                    # safely after the itB read). sA runs on PE while the
                    # B softmax + cK-B scale on Act/DVE/Pool.
                    emit_s_matmuls(qt, scaled_k, list(range(NA)),
                                   start=True, stop=False)
                    emit_softmax_half((NA, 12), r)
                    scale_group(NGA)        # [8,9]
                    scale_group(NGA + 1)    # [10,11]
                    emit_softmax_half((12, NCHUNK), r)
                    for gi in range(NGA + 2, len(all_groups)):
                        scale_group(gi)
                    if r == 0:
                        # save bias = incr for the next iteration; Act has
                        # slack here and reads psum fine
                        nc.scalar.activation(
                            bias[:].rearrange("p c j -> p (c j)"),
                            incr_ps[:], AF.Copy)
                    emit_s_matmuls(qt, scaled_k,
                                   list(range(NA, NCHUNK)),
                                   start=False, stop=True)

                    final = (r == ROUTING_STEPS - 1)
                    if final and comm:
                        # ReduceScatter: core c gets batches c*8..(c+1)*8
                        emit_evac(qt[0:B, :])
                        nc.gpsimd.collective_compute(
                            "ReduceScatter", OP.add, replica_groups=RG,
                            ins=[arin_d[:]], outs=[rsout_d[:]])
                        s_sh = once.tile([BSH, JE], fAR, tag="s_sh")
                        nc.sync.dma_start(s_sh[:], rsout_d[:])
                        emit_squash(1.0, BSH, s_sh, final=True)
                    else:
                        emit_allreduce(qt[0:B, :])
                        emit_squash(1.0, B, s_full, final=False)
    nc.compile()
    return nc


def _shard_inputs(inputs, kern):
    """Build the 8 per-core input maps (numpy preprocessing, fp16)."""
    # tail path: sel8 d-sums within each 16-row i-block and replicates
    sel8 = np.zeros((128, 128), dtype=np.float16)
    for i8 in range(ISUB):
        sel8[i8 * D:(i8 + 1) * D, i8 * D:(i8 + 1) * D] = 1.0
    # d-sum stationaries: selfull[(i8,d), c, q] = 1 iff q == 8c + i8
    selfull = np.zeros((128, 16, 128), dtype=np.float16)
    # broadcast stationaries: bmask[8c+i8, c, (i8,d)] = 1 replicates
    # incr_t row 8c+i8 across the d-partitions of chunk c's crep block
    bmask = np.zeros((128, 16, 128), dtype=np.float16)
    for c in range(16):
        for i8 in range(ISUB):
            for d in range(D):
                selfull[i8 * D + d, c, 8 * c + i8] = 1.0
                bmask[8 * c + i8, c, i8 * D + d] = 1.0
    cst = np.concatenate(
        [sel8, selfull.reshape(128, 2048), bmask.reshape(128, 2048)], axis=1)
    cst = np.ascontiguousarray(cst, dtype=np.float16)

    in_maps = []
    for c in range(N_CORES):
        lo, hi = c * I_LOC, (c + 1) * I_LOC
        x = np.ascontiguousarray(
            inputs[:, lo:hi, :].reshape(B, ID), dtype=np.float16)
        xt = np.ascontiguousarray(x.T)
        # K with (e, j) innermost: [(i,d), (e,j)]
        kk = np.ascontiguousarray(
            kern[lo:hi].transpose(0, 2, 3, 1).reshape(ID, JE),
            dtype=np.float16)
        in_maps.append({"x": x, "xt": xt, "kk": kk, "cst": cst})
    return in_maps


def kernel(inputs, kernel):
    import time

    from concourse.bass_utils import run_bass_kernel_spmd

    in_maps = _shard_inputs(np.asarray(inputs), np.asarray(kernel))
    last_err = None
    for attempt in range(3):
        try:
            if "nc" not in _CACHE:
                _CACHE["nc"] = _build_nc(repeat=1, ar_f32=AR_F32)
            res = run_bass_kernel_spmd(_CACHE["nc"], in_maps,
                                       list(range(N_CORES)))
            out = np.concatenate(
                [res.results[c]["out"] for c in range(N_CORES)], axis=0)
            return out.reshape(B, J, E).astype(np.float32)
        except Exception as e:  # transient NRT/device hiccups
            last_err = e
            _CACHE.pop("nc", None)
            try:
                import jax
                jax.clear_caches()
            except Exception:
                pass
            time.sleep(2.0 * (attempt + 1))
    raise last_err


# revision 38
# speedup vs baseline: 1.0273x; 1.0273x over previous
"""Trainium2 Bass kernel for CapsuleLayer dynamic routing (fp16, pipelined).

Math (reference):
    u_hat[b,i,j,e] = sum_d inputs[b,i,d] * kernel[i,j,d,e]
    3 routing iterations over shared bias[i,j] (softmax over j),
    s[b,j,e] = sum_i c[i,j] u_hat[b,i,j,e]; outputs = squash(s)
    bias += sum_{b,e} u_hat * outputs

Strategy: shard i (in_caps=1152 -> 144/core) across 8 cores; u_hat never
materialized.  All data SBUF-resident in fp16.  K stored with (e,j)
innermost so the c[i,j] broadcast lands on a middle dim (DVE 2x mode
needs unit-stride last dims).

Per routing iteration (A = chunks 0..7, B = 8..15, tails 16,17 first):
    per chunk: G = X^T O (PE, psum) -> fp16 copy (Act) -> P = K (*) G
    (DVE fp16 2x) -> d-sum on PE into row-halves of one psum tile Qt
    (A -> rows 0..63 via selfull[:, c, :64], B -> rows 64..127).
    Tail chunks take a DVE e-reduce + sel8-matmul path and are processed
    first so their serial reduces hide under the main pipeline.
    A's half closes right after chunk 8's G/P (e-reduce A -> bmask
    replicate -> bias/softmax for chunk-cols 0..7 -> cK-A scale) so the
    s matmuls for chunks 0..7 run right after the B d-sum burst while
    the B softmax + cK-B scale are still on Act/DVE.  B's Q matmuls
    trail their P by one chunk and keep bmA ahead of the B group-open.
    s accumulates into Qt rows 0..B (A data dead, B rows read by itB).
    AllReduce(s) fp16; squash -> O.  A junk matmul gated on s_full
    re-ramps the PE out of its idle pstate during each squash.
Final iteration: ReduceScatter, each core squashes + emits 8 batches.

The softmax over j is per (i, chunk-col) so the A/B split is exact.
NOTE pool_ck=True (cK scale groups on gpsimd) is OFF: the CoreSim cost
model prices Pool tensor_tensor at ~0.83ns/elem but on HW it measured
~16us/iter SLOWER (within-run A/B 189.8us vs 157.4us) — gpsimd is not
usable for bulk elementwise work.

Measured (HW repeat-delta, within one process): legacy 184,675 ns ->
this version 157,376 ns; standalone test.py run: 164,617 ns, rel err
2.291e-3 (gate 2e-2).  Cross-process numbers carry ~5-10us of
dispatch-anchor noise; only within-run comparisons are reliable.
"""

import sys

import numpy as np

if "/opt/trn_rl_repo" not in sys.path:
    sys.path.insert(0, "/opt/trn_rl_repo")

B, I, D, J, E = 64, 1152, 16, 32, 32
N_CORES = 8
I_LOC = I // N_CORES            # 144
ID = I_LOC * D                  # 2304
NCHUNK = ID // 128              # 18
NMAIN = 16                      # d-sum-first chunks (full-stationary masks)
NA = 8                          # A-half main chunks (0..7); B = 8..15 + tails
ISUB = 128 // D                 # 8 distinct i per 128-row chunk
JE = J * E                      # 1024
BSH = B // N_CORES              # 8 output batches per core
EPS = 1e-7
ROUTING_STEPS = 2               # routing iters after the uniform-c step

_CACHE = {}
AR_F32 = False  # fp16 collectives halve AR payload


def _build_nc(repeat=1, comm=True, ar_f32=False, pool_ck=False,
              direct_p=()):
    import concourse.mybir as mybir
    import concourse.tile as tile
    from concourse import bacc

    f32 = mybir.dt.float32
    f16 = mybir.dt.float16
    AX = mybir.AxisListType
    OP = mybir.AluOpType
    AF = mybir.ActivationFunctionType

    nc = bacc.Bacc("TRN2", target_bir_lowering=False, debug=False,
                   num_devices=N_CORES)
    x_d = nc.dram_tensor("x", [B, ID], f16, kind="ExternalInput")
    xt_d = nc.dram_tensor("xt", [ID, B], f16, kind="ExternalInput")
    k_d = nc.dram_tensor("kk", [ID, JE], f16, kind="ExternalInput")
    # packed constants: [sel8(128) | selfull(16*128) | bmask(16*128)]
    cst_d = nc.dram_tensor("cst", [128, 4224], f16, kind="ExternalInput")
    out_d = nc.dram_tensor("out", [BSH, JE], f32, kind="ExternalOutput")
    fAR = f32 if ar_f32 else f16
    arin_d = nc.dram_tensor("ar_in", [B, JE], fAR)
    arout_d = nc.dram_tensor("ar_out", [B, JE], fAR, addr_space="Shared")
    rsout_d = nc.dram_tensor("rs_out", [BSH, JE], fAR)
    RG = [list(range(N_CORES))]

    # cK scale groups; "pool" groups ride the otherwise-idle gpsimd
    # (sim Pool TT ~0.83ns/elem vs DVE-2x 0.55 — worth a big share)
    if pool_ck:
        CK_A = [([0, 1], "dve"), ([2, 3, 4], "pool"), ([5, 6, 7], "pool")]
        CK_B = [([8, 9], "dve"), ([10, 11], "dve"), ([12, 13], "dve"),
                ([14, 15, 16, 17], "pool")]
    else:
        CK_A = [([0, 1], "dve"), ([2, 3, 4], "dve"), ([5, 6, 7], "dve")]
        # B groups split at chunk 12: softmax for cols 8..11 closes first
        # so cK [8,9]/[10,11] start while cols 12..17 still in exp/sum
        CK_B = [([8, 9], "dve"), ([10, 11], "dve"), ([12, 13], "dve"),
                ([14, 15], "dve"), ([16, 17], "dve")]

    with tile.TileContext(nc) as tc:
        with (
            tc.tile_pool(name="big", bufs=1) as big,
            tc.tile_pool(name="work", bufs=2) as work,
            tc.tile_pool(name="pwork", bufs=1) as pwork,
            tc.tile_pool(name="gwork", bufs=6) as gwork,
            tc.tile_pool(name="once", bufs=1) as once,
            tc.tile_pool(name="small", bufs=2) as small,
            tc.tile_pool(name="gps", bufs=2, space="PSUM") as gps,
            tc.tile_pool(name="sps", bufs=1, space="PSUM") as sps,
            tc.tile_pool(name="qps", bufs=1, space="PSUM") as qps,
        ):
            # ---- resident inputs; few big DMAs (SP issuance ~600ns each)
            ksb = big.tile([128, NCHUNK, JE], f16)
            xtsb = big.tile([128, NCHUNK, B], f16)
            xsb = big.tile([B, ID], f16)
            cst = big.tile([128, 4224], f16)
            nc.sync.dma_start(
                xtsb[:], xt_d[:].rearrange("(c p) n -> p c n", p=128))
            nc.sync.dma_start(xsb[:], x_d[:])
            for g in range(3):
                nc.sync.dma_start(
                    ksb[:, g * 6:(g + 1) * 6, :],
                    k_d[:].rearrange("(c p) n -> p c n",
                                     p=128)[:, g * 6:(g + 1) * 6, :])
            nc.sync.dma_start(cst[:], cst_d[:])
            sel8 = cst[:, 0:128]
            selfull = cst[:, 128:2176].rearrange("p (m q) -> p m q", q=128)
            bmask = cst[:, 2176:4224].rearrange("p (m q) -> p m q", q=128)

            epsb = big.tile([B, 1], f32)
            nc.vector.memset(epsb[:], EPS)

            # preload the one act table serving copy/square/ln/exp so the
            # greedy per-func table chooser stops thrashing (~1.4us/load)
            import bass_rust
            _atl = bass_rust.InstLoadActFuncSet(
                name=nc.get_next_instruction_name(),
                act_func_set_id=6, ins=[], outs=[])
            nc.scalar.add_instruction(_atl)

            bias = big.tile([128, NCHUNK, J], f32)
            crep = big.tile([128, NCHUNK, J], f16)
            s_full = big.tile([B, JE], fAR)
            orr = big.tile([B, JE], f16)

            def emit_squash(alpha, nb, s_in, final):
                # squash(alpha * s_in) over e; layout [(b), (e j)]
                # square on Act (DVE is the busier engine); alpha**2 folded
                # into the tiny per-j n2 when alpha != 1
                # s*s on DVE fp16 2x: shortest serial chain (this path has
                # every engine idle, so chain length beats engine balance)
                # square + e-norm in column halves so half 0 proceeds while
                # half 1's return DMA is still landing
                sq = once.tile([B, JE], f16, tag="sq")
                m2h = small.tile([B, 2, J], f32, tag="m2h")
                for h in range(2):
                    sl = slice(h * 512, (h + 1) * 512)
                    with nc.allow_low_precision("fp16 squash"):
                        nc.vector.tensor_tensor(sq[:nb, sl], s_in[:nb, sl],
                                                s_in[:nb, sl], op=OP.mult)
                    nc.vector.tensor_reduce(
                        m2h[:nb, h, :],
                        sq[:nb, sl].rearrange("b (e j) -> b j e", j=J),
                        axis=AX.X, op=OP.add)
                m2 = small.tile([B, J], f32, tag="m2")
                nc.vector.tensor_tensor(m2[:nb, :], m2h[:nb, 0, :],
                                        m2h[:nb, 1, :], op=OP.add)
                if alpha != 1.0:
                    n2 = small.tile([B, J], f32, tag="n2")
                    nc.vector.tensor_scalar_mul(n2[:nb, :], m2[:nb, :],
                                                alpha * alpha)
                else:
                    n2 = m2
                # 1/sqrt(n2+eps) = exp(-0.5*ln(n2+eps)): keeps Act inside
                # one function table (exp/ln/copy/square)
                lg = small.tile([B, J], f32, tag="lg")
                nc.scalar.activation(lg[:nb, :], n2[:nb, :], AF.Ln,
                                     bias=epsb[:nb, :])
                rsq = small.tile([B, J], f32, tag="rsq")
                nc.scalar.activation(rsq[:nb, :], lg[:nb, :], AF.Exp,
                                     scale=-0.5)
                d1 = small.tile([B, J], f32, tag="d1")
                nc.vector.tensor_scalar_add(d1[:nb, :], n2[:nb, :], 1.0)
                rcp = small.tile([B, J], f32, tag="rcp")
                nc.vector.reciprocal(rcp[:nb, :], d1[:nb, :])
                if alpha != 1.0:
                    n2s = small.tile([B, J], f32, tag="n2s")
                    nc.vector.tensor_scalar_mul(n2s[:nb, :], n2[:nb, :],
                                                alpha)
                else:
                    n2s = n2
                fac0 = small.tile([B, J], f32, tag="fac0")
                nc.vector.tensor_tensor(fac0[:nb, :], n2s[:nb, :],
                                        rsq[:nb, :], op=OP.mult)
                # fp16 factor -> the big output multiply runs in DVE 2x mode
                facf = small.tile([B, J], f16, tag="facf")
                with nc.allow_low_precision("fp16 squash"):
                    nc.vector.tensor_tensor(facf[:nb, :], fac0[:nb, :],
                                            rcp[:nb, :], op=OP.mult)
                fb = facf[:nb, None, :].broadcast_to([nb, E, J])
                s3 = s_in[:nb, :].rearrange("b (e j) -> b e j", j=J)
                if final:
                    osb = once.tile([BSH, JE], f32, tag="osb")
                    with nc.allow_low_precision("fp16 squash"):
                        nc.vector.tensor_tensor(
                            osb[:].rearrange("b (j e) -> b e j", e=E),
                            s3, fb, op=OP.mult)
                    nc.sync.dma_start(out_d[:], osb[:])
                else:
                    with nc.allow_low_precision("fp16 squash"):
                        nc.vector.tensor_tensor(
                            orr[:nb, :].rearrange("b (e j) -> b e j", j=J),
                            s3, fb, op=OP.mult)

            def emit_evac(s_ps):
                # psum -> SBUF -> arin_d in column halves: half 0's DMA
                # overlaps half 1's Act copy
                s_sb = once.tile([B, JE], fAR, tag="s_sb")
                for h in range(2):
                    sl = slice(h * 512, (h + 1) * 512)
                    nc.scalar.activation(s_sb[:, sl], s_ps[:, sl], AF.Copy)
                    nc.sync.dma_start(arin_d[:, sl], s_sb[:, sl])

            def emit_allreduce(s_ps):
                emit_evac(s_ps)
                if comm:
                    nc.gpsimd.collective_compute(
                        "AllReduce", OP.add, replica_groups=RG,
                        ins=[arin_d[:]], outs=[arout_d[:]])
                    src = arout_d
                else:
                    src = arin_d
                # return in halves: half 0's squash ops overlap half 1
                for h in range(2):
                    sl = slice(h * 512, (h + 1) * 512)
                    nc.sync.dma_start(s_full[:, sl], src[:, sl])

            def emit_pe_warmup():
                # PE drops to low pstate during the collective; a junk
                # matmul gated on s_full's arrival ramps it back up so the
                # first real G matmuls run at speed. Output is discarded
                # (scratch psum bank reused each time).
                wps = gps.tile([128, JE], f32, tag="g")
                for h in range(2):
                    nc.tensor.matmul(
                        wps[:, h * 512:(h + 1) * 512],
                        s_full[:, 0:128], s_full[:, h * 512:(h + 1) * 512],
                        start=True, stop=True)

            def emit_s_matmuls(s_ps, rhs_of_chunk, chunks, start, stop):
                for n, c in enumerate(chunks):
                    rhs = rhs_of_chunk(c)
                    for h in range(2):
                        nc.tensor.matmul(
                            s_ps[0:B, h * 512:(h + 1) * 512],
                            xtsb[:, c, :],
                            rhs[:, h * 512:(h + 1) * 512],
                            start=(start and n == 0),
                            stop=(stop and n == len(chunks) - 1))

            for _rep in range(repeat):
                # ---- phase 0: s0 = X @ K (uniform c folded via alpha=1/J)
                s_ps = qps.tile([128, JE], f32, tag="qt")
                emit_s_matmuls(s_ps, lambda c: ksb[:, c, :],
                               list(range(NCHUNK)), True, True)
                emit_allreduce(s_ps[0:B, :])
                emit_squash(1.0 / J, B, s_full, final=False)

                for r in range(ROUTING_STEPS):
                    emit_pe_warmup()
                    qt = qps.tile([128, JE], f32, tag="qt")
                    incr_ps = sps.tile([128, NCHUNK * J], f32, tag="incr")
                    exe = once.tile([128, NCHUNK, J], f32, tag="exe")
                    it2 = once.tile([128, J], f16, tag="it2")
                    sm = small.tile([128, NCHUNK], f32, tag="sm")
                    rc = small.tile([128, NCHUNK], f32, tag="rc")
                    kps = {}
                    all_groups = ([(tuple(g), eng, f"kpA{n}")
                                   for n, (g, eng) in enumerate(CK_A)] +
                                  [(tuple(g), eng, f"kpB{n}")
                                   for n, (g, eng) in enumerate(CK_B)])
                    NGA = len(CK_A)

                    def scale_group(gi, _kps=kps):
                        grp, eng, tg = all_groups[gi]
                        if grp in _kps:
                            return
                        w, lo = len(grp), grp[0]
                        kp = work.tile([128, w, JE], f16, tag=tg)
                        engine = nc.vector if eng == "dve" else nc.gpsimd
                        with nc.allow_low_precision("fp16 cK"):
                            engine.tensor_tensor(
                                kp[:].rearrange("p c (e j) -> p c e j", j=J),
                                ksb[:, lo:lo + w, :].rearrange(
                                    "p c (e j) -> p c e j", j=J),
                                crep[:, lo:lo + w, None, :]
                                .broadcast_to([128, w, E, J]),
                                op=OP.mult)
                        _kps[grp] = kp

                    def scaled_k(c, _kps=kps):
                        for grp, eng, tg in all_groups:
                            if c in grp:
                                return _kps[grp][:, c - grp[0], :]
                        raise AssertionError(c)

                    def emit_softmax_half(cols, r):
                        # cols: (lo, hi) chunk-col range of bias/softmax
                        lo, hi = cols
                        bfl = bias[:].rearrange("p c j -> p (c j)")
                        if r > 0:
                            nc.vector.tensor_tensor(
                                bfl[:, lo * J:hi * J],
                                bfl[:, lo * J:hi * J],
                                incr_ps[:, lo * J:hi * J], op=OP.add)
                            src = bias[:, lo:hi, :]
                        else:
                            # exp straight off psum; bias saved later (DVE)
                            src = incr_ps[:, lo * J:hi * J].rearrange(
                                "p (c j) -> p c j", j=J)
                        nc.scalar.activation(exe[:, lo:hi, :], src, AF.Exp)
                        nc.vector.tensor_reduce(
                            sm[:, lo:hi], exe[:, lo:hi, :], axis=AX.X,
                            op=OP.add)
                        nc.vector.reciprocal(rc[:, lo:hi], sm[:, lo:hi])
                        with nc.allow_low_precision("fp16 c"):
                            nc.vector.tensor_tensor(
                                crep[:, lo:hi, :], exe[:, lo:hi, :],
                                rc[:, lo:hi, None]
                                .broadcast_to([128, hi - lo, J]),
                                op=OP.mult)

                    def emit_close_A():
                        # e-reduce rows 0..63, replicate via bmask,
                        # softmax for chunk-cols 0..7, first cK-A group.
                        # Emitted after main chunk 9 so neither PE (bmA
                        # waits itA) nor Act (expA waits bmA) stalls.
                        with nc.allow_low_precision("fp16 incr"):
                            nc.vector.tensor_reduce(
                                it2[0:64, :],
                                qt[0:64, :].rearrange(
                                    "p (e j) -> p j e", j=J),
                                axis=AX.X, op=OP.add)
                        for cc in range(NA):
                            nc.tensor.matmul(
                                incr_ps[:, cc * J:(cc + 1) * J],
                                bmask[0:64, cc, :], it2[0:64, :],
                                start=True, stop=True)
                        emit_softmax_half((0, NA), r)
                        for gi in range(NGA):
                            scale_group(gi)   # pool group runs concurrent

                    # ---- increments: tails first, then A (0..7), B (8..15)
                    # DIRECT_P chunks skip the Act copy: DVE multiplies K
                    # against the fp32 psum G directly (1x mode) — trades
                    # idle DVE for Act-pipe pace, and {14,15} unhook the
                    # pipe tail from the Act queue so itB closes earlier.
                    DIRECT_P = set(direct_p)

                    def emit_gcp(c):
                        # G = X^T O (PE) -> [fp16 copy (Act) ->] P (DVE)
                        g_ps = gps.tile([128, JE], f32, tag="g")
                        for h in range(2):
                            nc.tensor.matmul(
                                g_ps[:, h * 512:(h + 1) * 512],
                                xsb[:, c * 128:(c + 1) * 128],
                                orr[:, h * 512:(h + 1) * 512],
                                start=True, stop=True)
                        psup = pwork.tile([128, JE], f16, tag=f"p{c}")
                        if c in DIRECT_P:
                            nc.vector.tensor_tensor(psup[:], ksb[:, c, :],
                                                    g_ps[:], op=OP.mult)
                        else:
                            g_sb = gwork.tile([128, JE], f16, tag="gsb")
                            nc.scalar.activation(g_sb[:], g_ps[:], AF.Copy)
                            nc.vector.tensor_tensor(psup[:], ksb[:, c, :],
                                                    g_sb[:], op=OP.mult)
                        return psup

                    pend_sel8 = []
                    for c in [16, 17] + list(range(NA)):
                        psup = emit_gcp(c)
                        if c >= NMAIN:  # tail chunk: e-reduce + sel8 later
                            per = small.tile([128, J], f16, tag=f"per{c}")
                            with nc.allow_low_precision("fp16 incr"):
                                nc.vector.tensor_reduce(
                                    per[:],
                                    psup[:].rearrange(
                                        "p (e j) -> p j e", j=J),
                                    axis=AX.X, op=OP.add)
                            pend_sel8.append((c, per))
                            continue
                        for h in range(2):
                            nc.tensor.matmul(
                                qt[0:64, h * 512:(h + 1) * 512],
                                selfull[:, c, 0:64],
                                psup[:, h * 512:(h + 1) * 512],
                                start=(c == 0), stop=(c == NA - 1))
                        if c == 3 and pend_sel8:
                            # tails' incr cols; PE slack mid-pipe
                            for tc_, per in pend_sel8:
                                nc.tensor.matmul(
                                    incr_ps[:, tc_ * J:(tc_ + 1) * J],
                                    sel8[:], per[:],
                                    start=True, stop=True)
                            pend_sel8 = []
                    # B half: G/copy/P stream; close-A (incl. the itA read
                    # of qt) lands right after P8 so bmA precedes Q8's
                    # group-open in PE order; Q_c trails by one chunk.
                    prev_psup = None
                    for c in range(NA, NMAIN):
                        psup = emit_gcp(c)
                        if c == NA:
                            emit_close_A()
                        else:
                            for h in range(2):
                                nc.tensor.matmul(
                                    qt[64:128, h * 512:(h + 1) * 512],
                                    selfull[:, c - 1, 64:128],
                                    prev_psup[:, h * 512:(h + 1) * 512],
                                    start=(c - 1 == NA), stop=False)
                        prev_psup = psup
                    for h in range(2):
                        nc.tensor.matmul(
                            qt[64:128, h * 512:(h + 1) * 512],
                            selfull[:, NMAIN - 1, 64:128],
                            prev_psup[:, h * 512:(h + 1) * 512],
                            start=False, stop=True)
                    # close B: e-reduce rows 64..127, replicate, softmax
                    # cols 8..17 (tails' cols were filled via sel8)
                    with nc.allow_low_precision("fp16 incr"):
                        nc.vector.tensor_reduce(
                            it2[64:128, :],
                            qt[64:128, :].rearrange(
                                "p (e j) -> p j e", j=J),
                            axis=AX.X, op=OP.add)
                    for cc in range(NA, NMAIN):
                        nc.tensor.matmul(
                            incr_ps[:, cc * J:(cc + 1) * J],
                            bmask[64:128, cc, :], it2[64:128, :],
                            start=True, stop=True)
                    # ---- s_{r+1} = X @ (c (x) K): s reuses qt rows 0..B
                    # (A-group data is dead, B-rows already read by itB —
                    # bmB precedes sA in PE order so the group-open is
                    # safely after the itB read). sA runs on PE while the
                    # B softmax + cK-B scale on Act/DVE/Pool.
                    emit_s_matmuls(qt, scaled_k, list(range(NA)),
                                   start=True, stop=False)
                    emit_softmax_half((NA, 12), r)
                    scale_group(NGA)        # [8,9]
                    scale_group(NGA + 1)    # [10,11]
                    emit_softmax_half((12, NCHUNK), r)
                    for gi in range(NGA + 2, len(all_groups)):
                        scale_group(gi)
                    if r == 0:
                        # save bias = incr for the next iteration; Act has
                        # slack here and reads psum fine
                        nc.scalar.activation(
                            bias[:].rearrange("p c j -> p (c j)"),
                            incr_ps[:], AF.Copy)
                    emit_s_matmuls(qt, scaled_k,
                                   list(range(NA, NCHUNK)),
                                   start=False, stop=True)

                    final = (r == ROUTING_STEPS - 1)
                    if final and comm:
                        # ReduceScatter: core c gets batches c*8..(c+1)*8
                        emit_evac(qt[0:B, :])
                        nc.gpsimd.collective_compute(
                            "ReduceScatter", OP.add, replica_groups=RG,
                            ins=[arin_d[:]], outs=[rsout_d[:]])
                        s_sh = once.tile([BSH, JE], fAR, tag="s_sh")
                        nc.sync.dma_start(s_sh[:], rsout_d[:])
                        emit_squash(1.0, BSH, s_sh, final=True)
                    else:
                        emit_allreduce(qt[0:B, :])
                        emit_squash(1.0, B, s_full, final=False)
    nc.compile()
    return nc


def _shard_inputs(inputs, kern):
    """Build the 8 per-core input maps (numpy preprocessing, fp16)."""
    # tail path: sel8 d-sums within each 16-row i-block and replicates
    sel8 = np.zeros((128, 128), dtype=np.float16)
    for i8 in range(ISUB):
        sel8[i8 * D:(i8 + 1) * D, i8 * D:(i8 + 1) * D] = 1.0
    # d-sum stationaries: selfull[(i8,d), c, q] = 1 iff q == 8c + i8
    selfull = np.zeros((128, 16, 128), dtype=np.float16)
    # broadcast stationaries: bmask[8c+i8, c, (i8,d)] = 1 replicates
    # incr_t row 8c+i8 across the d-partitions of chunk c's crep block
    bmask = np.zeros((128, 16, 128), dtype=np.float16)
    for c in range(16):
        for i8 in range(ISUB):
            for d in range(D):
                selfull[i8 * D + d, c, 8 * c + i8] = 1.0
                bmask[8 * c + i8, c, i8 * D + d] = 1.0
    cst = np.concatenate(
        [sel8, selfull.reshape(128, 2048), bmask.reshape(128, 2048)], axis=1)
    cst = np.ascontiguousarray(cst, dtype=np.float16)

    in_maps = []
    for c in range(N_CORES):
        lo, hi = c * I_LOC, (c + 1) * I_LOC
        x = np.ascontiguousarray(
            inputs[:, lo:hi, :].reshape(B, ID), dtype=np.float16)
        xt = np.ascontiguousarray(x.T)
        # K with (e, j) innermost: [(i,d), (e,j)]
        kk = np.ascontiguousarray(
            kern[lo:hi].transpose(0, 2, 3, 1).reshape(ID, JE),
            dtype=np.float16)
        in_maps.append({"x": x, "xt": xt, "kk": kk, "cst": cst})
    return in_maps


def kernel(inputs, kernel):
    import time

    from concourse.bass_utils import run_bass_kernel_spmd

    in_maps = _shard_inputs(np.asarray(inputs), np.asarray(kernel))
    last_err = None
    for attempt in range(3):
        try:
            if "nc" not in _CACHE:
                _CACHE["nc"] = _build_nc(repeat=1, ar_f32=AR_F32)
            res = run_bass_kernel_spmd(_CACHE["nc"], in_maps,
                                       list(range(N_CORES)))
            out = np.concatenate(
                [res.results[c]["out"] for c in range(N_CORES)], axis=0)
            return out.reshape(B, J, E).astype(np.float32)
        except Exception as e:  # transient NRT/device hiccups
            last_err = e
            _CACHE.pop("nc", None)
            try:
                import jax
                jax.clear_caches()
            except Exception:
                pass
            time.sleep(2.0 * (attempt + 1))
    raise last_err


# revision 39
# speedup vs baseline: 1.0542x; 1.0262x over previous
"""Trainium2 Bass kernel for CapsuleLayer dynamic routing (fp16, pipelined).

Math (reference):
    u_hat[b,i,j,e] = sum_d inputs[b,i,d] * kernel[i,j,d,e]
    3 routing iterations over shared bias[i,j] (softmax over j),
    s[b,j,e] = sum_i c[i,j] u_hat[b,i,j,e]; outputs = squash(s)
    bias += sum_{b,e} u_hat * outputs

Strategy: shard i (in_caps=1152 -> 144/core) across 8 cores; u_hat never
materialized.  All data SBUF-resident in fp16.  K stored with (e,j)
innermost so the c[i,j] broadcast lands on a middle dim (DVE 2x mode
needs unit-stride last dims).

Per routing iteration (A = chunks 0..7, B = 8..15, tails 16,17 first):
    per chunk: G = X^T O (PE, psum) -> fp16 copy (Act) -> P = K (*) G
    (DVE fp16 2x) -> d-sum on PE into row-halves of one psum tile Qt
    (A -> rows 0..63 via selfull[:, c, :64], B -> rows 64..127).
    Tail chunks take a DVE e-reduce + sel8-matmul path and are processed
    first so their serial reduces hide under the main pipeline.
    A's half closes right after chunk 8's G/P (e-reduce A -> bmask
    replicate -> bias/softmax for chunk-cols 0..7 -> cK-A scale) so the
    s matmuls for chunks 0..7 run right after the B d-sum burst while
    the B softmax + cK-B scale are still on Act/DVE.  B's Q matmuls
    trail their P by one chunk and keep bmA ahead of the B group-open.
    s accumulates into Qt rows 0..B (A data dead, B rows read by itB).
    AllReduce(s) fp16; squash -> O.  A junk matmul gated on s_full
    re-ramps the PE out of its idle pstate during each squash.
Final iteration: ReduceScatter, each core squashes + emits 8 batches.

The softmax over j is per (i, chunk-col) so the A/B split is exact.
NOTE pool_ck=True (cK scale groups on gpsimd) is OFF: the CoreSim cost
model prices Pool tensor_tensor at ~0.83ns/elem but on HW it measured
~16us/iter SLOWER (within-run A/B 189.8us vs 157.4us) — gpsimd is not
usable for bulk elementwise work.

Measured (HW repeat-delta, within one process): legacy 184,675 ns ->
this version 157,376 ns; standalone test.py run: 164,617 ns, rel err
2.291e-3 (gate 2e-2).  Cross-process numbers carry ~5-10us of
dispatch-anchor noise; only within-run comparisons are reliable.
"""

import sys

import numpy as np

if "/opt/trn_rl_repo" not in sys.path:
    sys.path.insert(0, "/opt/trn_rl_repo")

B, I, D, J, E = 64, 1152, 16, 32, 32
N_CORES = 8
I_LOC = I // N_CORES            # 144
ID = I_LOC * D                  # 2304
NCHUNK = ID // 128              # 18
NMAIN = 16                      # d-sum-first chunks (full-stationary masks)
NA = 8                          # A-half main chunks (0..7); B = 8..15 + tails
ISUB = 128 // D                 # 8 distinct i per 128-row chunk
JE = J * E                      # 1024
BSH = B // N_CORES              # 8 output batches per core
EPS = 1e-7
ROUTING_STEPS = 2               # routing iters after the uniform-c step

_CACHE = {}
AR_F32 = False  # fp16 collectives halve AR payload


def _build_nc(repeat=1, comm=True, ar_f32=False, pool_ck=False,
              direct_p=()):
    import concourse.mybir as mybir
    import concourse.tile as tile
    from concourse import bacc

    f32 = mybir.dt.float32
    f16 = mybir.dt.float16
    AX = mybir.AxisListType
    OP = mybir.AluOpType
    AF = mybir.ActivationFunctionType

    nc = bacc.Bacc("TRN2", target_bir_lowering=False, debug=False,
                   num_devices=N_CORES)
    x_d = nc.dram_tensor("x", [B, ID], f16, kind="ExternalInput")
    xt_d = nc.dram_tensor("xt", [ID, B], f16, kind="ExternalInput")
    k_d = nc.dram_tensor("kk", [ID, JE], f16, kind="ExternalInput")
    # packed constants: [sel8(128) | selfull(16*128) | bmask(16*128)]
    cst_d = nc.dram_tensor("cst", [128, 4224], f16, kind="ExternalInput")
    out_d = nc.dram_tensor("out", [BSH, JE], f32, kind="ExternalOutput")
    fAR = f32 if ar_f32 else f16
    arin_d = nc.dram_tensor("ar_in", [B, JE], fAR)
    arout_d = nc.dram_tensor("ar_out", [B, JE], fAR, addr_space="Shared")
    rsout_d = nc.dram_tensor("rs_out", [BSH, JE], fAR)
    RG = [list(range(N_CORES))]

    # cK scale groups; "pool" groups ride the otherwise-idle gpsimd
    # (sim Pool TT ~0.83ns/elem vs DVE-2x 0.55 — worth a big share)
    if pool_ck:
        CK_A = [([0, 1], "dve"), ([2, 3, 4], "pool"), ([5, 6, 7], "pool")]
        CK_B = [([8, 9], "dve"), ([10, 11], "dve"), ([12, 13], "dve"),
                ([14, 15, 16, 17], "pool")]
    else:
        CK_A = [([0, 1], "dve"), ([2, 3, 4], "dve"), ([5, 6, 7], "dve")]
        # B groups split at chunk 12: softmax for cols 8..11 closes first
        # so cK [8,9]/[10,11] start while cols 12..17 still in exp/sum
        CK_B = [([8, 9], "dve"), ([10, 11], "dve"), ([12, 13], "dve"),
                ([14, 15], "dve"), ([16, 17], "dve")]

    with tile.TileContext(nc) as tc:
        with (
            tc.tile_pool(name="big", bufs=1) as big,
            tc.tile_pool(name="work", bufs=2) as work,
            tc.tile_pool(name="pwork", bufs=1) as pwork,
            tc.tile_pool(name="gwork", bufs=6) as gwork,
            tc.tile_pool(name="once", bufs=1) as once,
            tc.tile_pool(name="small", bufs=2) as small,
            tc.tile_pool(name="gps", bufs=2, space="PSUM") as gps,
            tc.tile_pool(name="sps", bufs=1, space="PSUM") as sps,
            tc.tile_pool(name="qps", bufs=1, space="PSUM") as qps,
        ):
            # ---- resident inputs; few big DMAs (SP issuance ~600ns each)
            ksb = big.tile([128, NCHUNK, JE], f16)
            xtsb = big.tile([128, NCHUNK, B], f16)
            xsb = big.tile([B, ID], f16)
            cst = big.tile([128, 4224], f16)
            nc.sync.dma_start(
                xtsb[:], xt_d[:].rearrange("(c p) n -> p c n", p=128))
            nc.sync.dma_start(xsb[:], x_d[:])
            for g in range(3):
                nc.sync.dma_start(
                    ksb[:, g * 6:(g + 1) * 6, :],
                    k_d[:].rearrange("(c p) n -> p c n",
                                     p=128)[:, g * 6:(g + 1) * 6, :])
            nc.sync.dma_start(cst[:], cst_d[:])
            sel8 = cst[:, 0:128]
            selfull = cst[:, 128:2176].rearrange("p (m q) -> p m q", q=128)
            bmask = cst[:, 2176:4224].rearrange("p (m q) -> p m q", q=128)

            epsb = big.tile([B, 1], f32)
            nc.vector.memset(epsb[:], EPS)

            # preload the one act table serving copy/square/ln/exp so the
            # greedy per-func table chooser stops thrashing (~1.4us/load)
            import bass_rust
            _atl = bass_rust.InstLoadActFuncSet(
                name=nc.get_next_instruction_name(),
                act_func_set_id=6, ins=[], outs=[])
            nc.scalar.add_instruction(_atl)

            bias = big.tile([128, NCHUNK, J], f32)
            crep = big.tile([128, NCHUNK, J], f16)
            s_full = big.tile([B, JE], fAR)
            orr = big.tile([B, JE], f16)

            def emit_squash(alpha, nb, s_in, final):
                # squash(alpha * s_in) over e; layout [(b), (e j)]
                # square on Act (DVE is the busier engine); alpha**2 folded
                # into the tiny per-j n2 when alpha != 1
                # s*s on DVE fp16 2x: shortest serial chain (this path has
                # every engine idle, so chain length beats engine balance)
                # square + e-norm in column halves so half 0 proceeds while
                # half 1's return DMA is still landing
                sq = once.tile([B, JE], f16, tag="sq")
                m2h = small.tile([B, 2, J], f32, tag="m2h")
                for h in range(2):
                    sl = slice(h * 512, (h + 1) * 512)
                    with nc.allow_low_precision("fp16 squash"):
                        nc.vector.tensor_tensor(sq[:nb, sl], s_in[:nb, sl],
                                                s_in[:nb, sl], op=OP.mult)
                    nc.vector.tensor_reduce(
                        m2h[:nb, h, :],
                        sq[:nb, sl].rearrange("b (e j) -> b j e", j=J),
                        axis=AX.X, op=OP.add)
                m2 = small.tile([B, J], f32, tag="m2")
                nc.vector.tensor_tensor(m2[:nb, :], m2h[:nb, 0, :],
                                        m2h[:nb, 1, :], op=OP.add)
                if alpha != 1.0:
                    n2 = small.tile([B, J], f32, tag="n2")
                    nc.vector.tensor_scalar_mul(n2[:nb, :], m2[:nb, :],
                                                alpha * alpha)
                else:
                    n2 = m2
                # 1/sqrt(n2+eps) = exp(-0.5*ln(n2+eps)): keeps Act inside
                # one function table (exp/ln/copy/square)
                lg = small.tile([B, J], f32, tag="lg")
                nc.scalar.activation(lg[:nb, :], n2[:nb, :], AF.Ln,
                                     bias=epsb[:nb, :])
                rsq = small.tile([B, J], f32, tag="rsq")
                nc.scalar.activation(rsq[:nb, :], lg[:nb, :], AF.Exp,
                                     scale=-0.5)
                d1 = small.tile([B, J], f32, tag="d1")
                nc.vector.tensor_scalar_add(d1[:nb, :], n2[:nb, :], 1.0)
                rcp = small.tile([B, J], f32, tag="rcp")
                nc.vector.reciprocal(rcp[:nb, :], d1[:nb, :])
                if alpha != 1.0:
                    n2s = small.tile([B, J], f32, tag="n2s")
                    nc.vector.tensor_scalar_mul(n2s[:nb, :], n2[:nb, :],
                                                alpha)
                else:
                    n2s = n2
                # t = n2*rcp on DVE overlaps Act's ln->exp; facf = t*rsq
                # (associativity: n2*rsq*rcp) — one fewer serial op
                t0 = small.tile([B, J], f32, tag="t0")
                nc.vector.tensor_tensor(t0[:nb, :], n2s[:nb, :],
                                        rcp[:nb, :], op=OP.mult)
                # fp16 factor -> the big output multiply runs in DVE 2x mode
                facf = small.tile([B, J], f16, tag="facf")
                with nc.allow_low_precision("fp16 squash"):
                    nc.vector.tensor_tensor(facf[:nb, :], t0[:nb, :],
                                            rsq[:nb, :], op=OP.mult)
                fb = facf[:nb, None, :].broadcast_to([nb, E, J])
                s3 = s_in[:nb, :].rearrange("b (e j) -> b e j", j=J)
                if final:
                    osb = once.tile([BSH, JE], f32, tag="osb")
                    with nc.allow_low_precision("fp16 squash"):
                        nc.vector.tensor_tensor(
                            osb[:].rearrange("b (j e) -> b e j", e=E),
                            s3, fb, op=OP.mult)
                    nc.sync.dma_start(out_d[:], osb[:])
                else:
                    with nc.allow_low_precision("fp16 squash"):
                        nc.vector.tensor_tensor(
                            orr[:nb, :].rearrange("b (e j) -> b e j", j=J),
                            s3, fb, op=OP.mult)

            def emit_evac(s_ps):
                # psum -> SBUF -> arin_d in column halves: half 0's DMA
                # overlaps half 1's Act copy
                s_sb = once.tile([B, JE], fAR, tag="s_sb")
                for h in range(2):
                    sl = slice(h * 512, (h + 1) * 512)
                    nc.scalar.activation(s_sb[:, sl], s_ps[:, sl], AF.Copy)
                    nc.sync.dma_start(arin_d[:, sl], s_sb[:, sl])

            def emit_allreduce(s_ps):
                emit_evac(s_ps)
                if comm:
                    nc.gpsimd.collective_compute(
                        "AllReduce", OP.add, replica_groups=RG,
                        ins=[arin_d[:]], outs=[arout_d[:]])
                    src = arout_d
                else:
                    src = arin_d
                # return in halves: half 0's squash ops overlap half 1
                for h in range(2):
                    sl = slice(h * 512, (h + 1) * 512)
                    nc.sync.dma_start(s_full[:, sl], src[:, sl])

            def emit_pe_warmup():
                # PE drops to low pstate during the collective; a junk
                # matmul gated on s_full's arrival ramps it back up so the
                # first real G matmuls run at speed. Output is discarded
                # (scratch psum bank reused each time).
                wps = gps.tile([128, JE], f32, tag="g")
                for h in range(2):
                    nc.tensor.matmul(
                        wps[:, h * 512:(h + 1) * 512],
                        s_full[:, 0:128], s_full[:, h * 512:(h + 1) * 512],
                        start=True, stop=True)

            def emit_s_matmuls(s_ps, rhs_of_chunk, chunks, start, stop):
                for n, c in enumerate(chunks):
                    rhs = rhs_of_chunk(c)
                    for h in range(2):
                        nc.tensor.matmul(
                            s_ps[0:B, h * 512:(h + 1) * 512],
                            xtsb[:, c, :],
                            rhs[:, h * 512:(h + 1) * 512],
                            start=(start and n == 0),
                            stop=(stop and n == len(chunks) - 1))

            for _rep in range(repeat):
                # ---- phase 0: s0 = X @ K (uniform c folded via alpha=1/J)
                s_ps = qps.tile([128, JE], f32, tag="qt")
                emit_s_matmuls(s_ps, lambda c: ksb[:, c, :],
                               list(range(NCHUNK)), True, True)
                emit_allreduce(s_ps[0:B, :])
                emit_squash(1.0 / J, B, s_full, final=False)

                for r in range(ROUTING_STEPS):
                    emit_pe_warmup()
                    qt = qps.tile([128, JE], f32, tag="qt")
                    incr_ps = sps.tile([128, NCHUNK * J], f32, tag="incr")
                    exe = once.tile([128, NCHUNK, J], f32, tag="exe")
                    it2 = once.tile([128, J], f16, tag="it2")
                    sm = small.tile([128, NCHUNK], f32, tag="sm")
                    rc = small.tile([128, NCHUNK], f32, tag="rc")
                    kps = {}
                    all_groups = ([(tuple(g), eng, f"kpA{n}")
                                   for n, (g, eng) in enumerate(CK_A)] +
                                  [(tuple(g), eng, f"kpB{n}")
                                   for n, (g, eng) in enumerate(CK_B)])
                    NGA = len(CK_A)

                    def scale_group(gi, _kps=kps):
                        grp, eng, tg = all_groups[gi]
                        if grp in _kps:
                            return
                        w, lo = len(grp), grp[0]
                        kp = work.tile([128, w, JE], f16, tag=tg)
                        engine = nc.vector if eng == "dve" else nc.gpsimd
                        with nc.allow_low_precision("fp16 cK"):
                            engine.tensor_tensor(
                                kp[:].rearrange("p c (e j) -> p c e j", j=J),
                                ksb[:, lo:lo + w, :].rearrange(
                                    "p c (e j) -> p c e j", j=J),
                                crep[:, lo:lo + w, None, :]
                                .broadcast_to([128, w, E, J]),
                                op=OP.mult)
                        _kps[grp] = kp

                    def scaled_k(c, _kps=kps):
                        for grp, eng, tg in all_groups:
                            if c in grp:
                                return _kps[grp][:, c - grp[0], :]
                        raise AssertionError(c)

                    def emit_softmax_half(cols, r):
                        # cols: (lo, hi) chunk-col range of bias/softmax
                        lo, hi = cols
                        bfl = bias[:].rearrange("p c j -> p (c j)")
                        if r > 0:
                            nc.vector.tensor_tensor(
                                bfl[:, lo * J:hi * J],
                                bfl[:, lo * J:hi * J],
                                incr_ps[:, lo * J:hi * J], op=OP.add)
                            src = bias[:, lo:hi, :]
                        else:
                            # exp straight off psum; bias saved later (DVE)
                            src = incr_ps[:, lo * J:hi * J].rearrange(
                                "p (c j) -> p c j", j=J)
                        nc.scalar.activation(exe[:, lo:hi, :], src, AF.Exp)
                        nc.vector.tensor_reduce(
                            sm[:, lo:hi], exe[:, lo:hi, :], axis=AX.X,
                            op=OP.add)
                        nc.vector.reciprocal(rc[:, lo:hi], sm[:, lo:hi])
                        with nc.allow_low_precision("fp16 c"):
                            nc.vector.tensor_tensor(
                                crep[:, lo:hi, :], exe[:, lo:hi, :],
                                rc[:, lo:hi, None]
                                .broadcast_to([128, hi - lo, J]),
                                op=OP.mult)

                    def emit_close_A():
                        # e-reduce rows 0..63, replicate via bmask,
                        # softmax for chunk-cols 0..7, first cK-A group.
                        # Emitted after main chunk 9 so neither PE (bmA
                        # waits itA) nor Act (expA waits bmA) stalls.
                        with nc.allow_low_precision("fp16 incr"):
                            nc.vector.tensor_reduce(
                                it2[0:64, :],
                                qt[0:64, :].rearrange(
                                    "p (e j) -> p j e", j=J),
                                axis=AX.X, op=OP.add)
                        for cc in range(NA):
                            nc.tensor.matmul(
                                incr_ps[:, cc * J:(cc + 1) * J],
                                bmask[0:64, cc, :], it2[0:64, :],
                                start=True, stop=True)
                        emit_softmax_half((0, NA), r)
                        for gi in range(NGA):
                            scale_group(gi)   # pool group runs concurrent

                    # ---- increments: tails first, then A (0..7), B (8..15)
                    # DIRECT_P chunks skip the Act copy: DVE multiplies K
                    # against the fp32 psum G directly (1x mode) — trades
                    # idle DVE for Act-pipe pace, and {14,15} unhook the
                    # pipe tail from the Act queue so itB closes earlier.
                    DIRECT_P = set(direct_p)

                    def emit_gcp(c):
                        # G = X^T O (PE) -> [fp16 copy (Act) ->] P (DVE)
                        g_ps = gps.tile([128, JE], f32, tag="g")
                        for h in range(2):
                            nc.tensor.matmul(
                                g_ps[:, h * 512:(h + 1) * 512],
                                xsb[:, c * 128:(c + 1) * 128],
                                orr[:, h * 512:(h + 1) * 512],
                                start=True, stop=True)
                        psup = pwork.tile([128, JE], f16, tag=f"p{c}")
                        if c in DIRECT_P:
                            nc.vector.tensor_tensor(psup[:], ksb[:, c, :],
                                                    g_ps[:], op=OP.mult)
                        else:
                            g_sb = gwork.tile([128, JE], f16, tag="gsb")
                            nc.scalar.activation(g_sb[:], g_ps[:], AF.Copy)
                            nc.vector.tensor_tensor(psup[:], ksb[:, c, :],
                                                    g_sb[:], op=OP.mult)
                        return psup

                    pend_sel8 = []
                    for c in [16, 17] + list(range(NA)):
                        psup = emit_gcp(c)
                        if c >= NMAIN:  # tail chunk: e-reduce + sel8 later
                            per = small.tile([128, J], f16, tag=f"per{c}")
                            with nc.allow_low_precision("fp16 incr"):
                                nc.vector.tensor_reduce(
                                    per[:],
                                    psup[:].rearrange(
                                        "p (e j) -> p j e", j=J),
                                    axis=AX.X, op=OP.add)
                            pend_sel8.append((c, per))
                            continue
                        for h in range(2):
                            nc.tensor.matmul(
                                qt[0:64, h * 512:(h + 1) * 512],
                                selfull[:, c, 0:64],
                                psup[:, h * 512:(h + 1) * 512],
                                start=(c == 0), stop=(c == NA - 1))
                        if c == 3 and pend_sel8:
                            # tails' incr cols; PE slack mid-pipe
                            for tc_, per in pend_sel8:
                                nc.tensor.matmul(
                                    incr_ps[:, tc_ * J:(tc_ + 1) * J],
                                    sel8[:], per[:],
                                    start=True, stop=True)
                            pend_sel8 = []
                    # B half: G/copy/P stream; close-A (incl. the itA read
                    # of qt) lands right after P8 so bmA precedes Q8's
                    # group-open in PE order; Q_c trails by one chunk.
                    prev_psup = None
                    for c in range(NA, NMAIN):
                        psup = emit_gcp(c)
                        if c == NA:
                            emit_close_A()
                        else:
                            for h in range(2):
                                nc.tensor.matmul(
                                    qt[64:128, h * 512:(h + 1) * 512],
                                    selfull[:, c - 1, 64:128],
                                    prev_psup[:, h * 512:(h + 1) * 512],
                                    start=(c - 1 == NA), stop=False)
                        prev_psup = psup
                    for h in range(2):
                        nc.tensor.matmul(
                            qt[64:128, h * 512:(h + 1) * 512],
                            selfull[:, NMAIN - 1, 64:128],
                            prev_psup[:, h * 512:(h + 1) * 512],
                            start=False, stop=True)
                    # close B: e-reduce rows 64..127, replicate, softmax
                    # cols 8..17 (tails' cols were filled via sel8)
                    with nc.allow_low_precision("fp16 incr"):
                        nc.vector.tensor_reduce(
                            it2[64:128, :],
                            qt[64:128, :].rearrange(
                                "p (e j) -> p j e", j=J),
                            axis=AX.X, op=OP.add)
                    for cc in range(NA, NMAIN):
                        nc.tensor.matmul(
                            incr_ps[:, cc * J:(cc + 1) * J],
                            bmask[64:128, cc, :], it2[64:128, :],
                            start=True, stop=True)
                    # ---- s_{r+1} = X @ (c (x) K): s reuses qt rows 0..B
                    # (A-group data is dead, B-rows already read by itB —
                    # bmB precedes sA in PE order so the group-open is
                    # safely after the itB read). sA runs on PE while the
                    # B softmax + cK-B scale on Act/DVE/Pool.
                    emit_s_matmuls(qt, scaled_k, list(range(NA)),
                                   start=True, stop=False)
                    emit_softmax_half((NA, 12), r)
                    scale_group(NGA)        # [8,9]
                    scale_group(NGA + 1)    # [10,11]
                    emit_softmax_half((12, NCHUNK), r)
                    for gi in range(NGA + 2, len(all_groups)):
                        scale_group(gi)
                    if r == 0:
                        # save bias = incr for the next iteration; Act has
                        # slack here and reads psum fine
                        nc.scalar.activation(
                            bias[:].rearrange("p c j -> p (c j)"),
                            incr_ps[:], AF.Copy)
                    emit_s_matmuls(qt, scaled_k,
                                   list(range(NA, NCHUNK)),
                                   start=False, stop=True)

                    final = (r == ROUTING_STEPS - 1)
                    if final and comm:
                        # ReduceScatter: core c gets batches c*8..(c+1)*8
                        emit_evac(qt[0:B, :])
                        nc.gpsimd.collective_compute(
                            "ReduceScatter", OP.add, replica_groups=RG,
                            ins=[arin_d[:]], outs=[rsout_d[:]])
                        s_sh = once.tile([BSH, JE], fAR, tag="s_sh")
                        nc.sync.dma_start(s_sh[:], rsout_d[:])
                        emit_squash(1.0, BSH, s_sh, final=True)
                    else:
                        emit_allreduce(qt[0:B, :])
                        emit_squash(1.0, B, s_full, final=False)
    nc.compile()
    return nc


def _shard_inputs(inputs, kern):
    """Build the 8 per-core input maps (numpy preprocessing, fp16)."""
    # tail path: sel8 d-sums within each 16-row i-block and replicates
    sel8 = np.zeros((128, 128), dtype=np.float16)
    for i8 in range(ISUB):
        sel8[i8 * D:(i8 + 1) * D, i8 * D:(i8 + 1) * D] = 1.0
    # d-sum stationaries: selfull[(i8,d), c, q] = 1 iff q == 8c + i8
    selfull = np.zeros((128, 16, 128), dtype=np.float16)
    # broadcast stationaries: bmask[8c+i8, c, (i8,d)] = 1 replicates
    # incr_t row 8c+i8 across the d-partitions of chunk c's crep block
    bmask = np.zeros((128, 16, 128), dtype=np.float16)
    for c in range(16):
        for i8 in range(ISUB):
            for d in range(D):
                selfull[i8 * D + d, c, 8 * c + i8] = 1.0
                bmask[8 * c + i8, c, i8 * D + d] = 1.0
    cst = np.concatenate(
        [sel8, selfull.reshape(128, 2048), bmask.reshape(128, 2048)], axis=1)
    cst = np.ascontiguousarray(cst, dtype=np.float16)

    in_maps = []
    for c in range(N_CORES):
        lo, hi = c * I_LOC, (c + 1) * I_LOC
        x = np.ascontiguousarray(
            inputs[:, lo:hi, :].reshape(B, ID), dtype=np.float16)
        xt = np.ascontiguousarray(x.T)
        # K with (e, j) innermost: [(i,d), (e,j)]
        kk = np.ascontiguousarray(
            kern[lo:hi].transpose(0, 2, 3, 1).reshape(ID, JE),
            dtype=np.float16)
        in_maps.append({"x": x, "xt": xt, "kk": kk, "cst": cst})
    return in_maps


def kernel(inputs, kernel):
    import time

    from concourse.bass_utils import run_bass_kernel_spmd

    in_maps = _shard_inputs(np.asarray(inputs), np.asarray(kernel))
    last_err = None
    for attempt in range(3):
        try:
            if "nc" not in _CACHE:
                _CACHE["nc"] = _build_nc(repeat=1, ar_f32=AR_F32)
            res = run_bass_kernel_spmd(_CACHE["nc"], in_maps,
                                       list(range(N_CORES)))
            out = np.concatenate(
                [res.results[c]["out"] for c in range(N_CORES)], axis=0)
            return out.reshape(B, J, E).astype(np.float32)
        except Exception as e:  # transient NRT/device hiccups
            last_err = e
            _CACHE.pop("nc", None)
            try:
                import jax
                jax.clear_caches()
            except Exception:
                pass
            time.sleep(2.0 * (attempt + 1))
    raise last_err
